# revision 1
# baseline (speedup 1.0000x reference)
"""Trainium2 Bass kernel for BeatPatternExtractor:
quantized conv1d (stride 2) -> training-mode BatchNorm -> ELU -> multi-scale
spiking window/global attention with residual.

Sharding: data-parallel over batch (32 samples -> 4 per core x 8 cores).
BN batch stats are combined with a 1KB on-device AllReduce.

Numerics:
- conv: x in fp32r (TF32-like) x sign(w) in fp32r, alpha/bias folded into the
  BN affine on the host -> ~1.6e-4 component error at full PE rate.
- projections (wq/wk/wv/wo): fp32r.
- spiking attention: exact. Spikes are {0,1} in bf16, attention weights are
  integer counts <= 128 (exact in bf16), PSUM accumulates in fp32. The /64,
  /256, /16 window scales are folded into spike/kv evacuation as exact powers
  of two, and the /3 is folded into wo on the host.
"""
import sys

sys.path.insert(0, "/opt/trn_rl_repo")

import numpy as np

import concourse.bass as bass  # noqa: F401  (engine classes referenced via nc)
import concourse.mybir as mybir
import concourse.tile as tile
from concourse import bacc
from concourse.bass_utils import run_bass_kernel_spmd
from concourse.masks import make_identity

dt = mybir.dt
AF = mybir.ActivationFunctionType
ALU = mybir.AluOpType

N_CORES = 8
B, CIN, L = 32, 256, 5000
COUT, KW = 128, 9
LOUT = 2500
BPC = B // N_CORES          # samples per core
LPAD = 2560                 # padded attention domain: 5 tiles of 512
NPOS = B * LOUT             # BN normalization count
CHUNK = 157                 # global-attn pooling chunk = ceil(2500/16)
GPOOL = 16
EPS = 1e-5

# conv L tiles: (start, count)
CONV_TILES = [(0, 512), (512, 512), (1024, 512), (1536, 512), (2048, 452)]
ATT_LT = 5                  # attention l-tiles of 512 over LPAD
POS_TILES = 20              # position-major tiles of 128 over LPAD


def _pool_segments():
    """(g, lt, col_in_tile, width) segments of each pooling chunk, split at
    512-wide projection-tile boundaries."""
    segs = []
    for g in range(GPOOL):
        s, e = CHUNK * g, min(CHUNK * (g + 1), LOUT)
        cur = s
        while cur < e:
            lt = cur // 512
            te = min(e, (lt + 1) * 512)
            segs.append((g, lt, cur - lt * 512, te - cur))
            cur = te
    return segs


def _build_kernel(dbg=False):
    nc = bacc.Bacc("TRN2", target_bir_lowering=False, debug=False,
                   num_devices=N_CORES)

    xh_d = nc.dram_tensor("xs_hi", [BPC, CIN, 2, LOUT], dt.bfloat16,
                          kind="ExternalInput")
    xl_d = nc.dram_tensor("xs_lo", [BPC, CIN, 2, LOUT], dt.bfloat16,
                          kind="ExternalInput")
    wconv_d = nc.dram_tensor("wconv", [KW, 2, 128, COUT], dt.bfloat16,
                             kind="ExternalInput")
    wproj_d = nc.dram_tensor("wproj", [128, 4 * 128], dt.float32,
                             kind="ExternalInput")
    wqk_d = nc.dram_tensor("wqk_hl", [128, 4 * 128], dt.bfloat16,
                           kind="ExternalInput")
    vecs_d = nc.dram_tensor("vecs", [128, 5], dt.float32, kind="ExternalInput")
    rows_d = nc.dram_tensor("rows", [128, 144], dt.float32, kind="ExternalInput")
    rows4_d = nc.dram_tensor("rows4", [128, 512], dt.float32,
                             kind="ExternalInput")
    yout_d = nc.dram_tensor("yout", [BPC, COUT, LOUT], dt.float32,
                            kind="ExternalOutput")
    dbg_d = None
    if dbg:
        dbg_d = {
            "conv0": nc.dram_tensor("d_conv0", [COUT, LOUT], dt.float32,
                                    kind="ExternalOutput"),
            "h0": nc.dram_tensor("d_h0", [COUT, LOUT], dt.float32,
                                 kind="ExternalOutput"),
            "bn": nc.dram_tensor("d_bn", [128, 4], dt.float32,
                                 kind="ExternalOutput"),
            "sq0": nc.dram_tensor("d_sq0", [COUT, LPAD], dt.bfloat16,
                                  kind="ExternalOutput"),
            "sk0": nc.dram_tensor("d_sk0", [COUT, LPAD], dt.bfloat16,
                                  kind="ExternalOutput"),
            "sv0": nc.dram_tensor("d_sv0", [128, LPAD], dt.bfloat16,
                                  kind="ExternalOutput"),
            "o20": nc.dram_tensor("d_o20", [COUT, LPAD], dt.float32,
                                  kind="ExternalOutput"),
        }

    with tile.TileContext(nc) as tc:
        _body(tc, nc, xh_d, xl_d, wconv_d, wproj_d, wqk_d, vecs_d, rows_d, rows4_d, yout_d, dbg_d)
    nc.compile()
    return nc


def _body(tc, nc, xh_d, xl_d, wconv_d, wproj_d, wqk_d, vecs_d, rows_d, rows4_d, yout_d, dbg_d=None):
    import contextlib
    ctx = contextlib.ExitStack()
    with ctx:
        const = ctx.enter_context(tc.tile_pool(name="const", bufs=1))
        xf_pool = ctx.enter_context(tc.tile_pool(name="xf", bufs=2))
        ysb_pool = ctx.enter_context(tc.tile_pool(name="ysb", bufs=1))
        stat_pool = ctx.enter_context(tc.tile_pool(name="stat", bufs=1))
        sqs_pool = ctx.enter_context(tc.tile_pool(name="sqs", bufs=2))
        bn_pool = ctx.enter_context(tc.tile_pool(name="bn", bufs=1))
        hr_pool = ctx.enter_context(tc.tile_pool(name="hr", bufs=2))
        spk_pool = ctx.enter_context(tc.tile_pool(name="spk", bufs=2))
        svp_pool = ctx.enter_context(tc.tile_pool(name="svp", bufs=2))
        pool_pool = ctx.enter_context(tc.tile_pool(name="pool", bufs=2))
        abf_pool = ctx.enter_context(tc.tile_pool(name="abf", bufs=3))
        o2_pool = ctx.enter_context(tc.tile_pool(name="o2", bufs=3))
        fin_pool = ctx.enter_context(tc.tile_pool(name="fin", bufs=2))
        tmp_pool = ctx.enter_context(tc.tile_pool(name="tmp", bufs=3))

        bigps = ctx.enter_context(tc.tile_pool(name="bigps", bufs=3, space="PSUM"))
        a256ps = ctx.enter_context(tc.tile_pool(name="a256ps", bufs=1, space="PSUM"))
        smps = ctx.enter_context(tc.tile_pool(name="smps", bufs=2, space="PSUM"))

        dram = ctx.enter_context(tc.tile_pool(name="dram", bufs=1, space="DRAM"))

        # ---------- constants / weights ----------
        wc_bf = const.tile([128, 18 * 128], dt.bfloat16, tag="wc_bf",
                           name="wc_bf")
        for k in range(KW):
            for ci in range(2):
                j = k * 2 + ci
                nc.sync.dma_start(wc_bf[:, j * 128:(j + 1) * 128],
                                  wconv_d.ap()[k, ci])

        wp_f = const.tile([128, 512], dt.float32, tag="wp_f", name="wp_f")
        nc.sync.dma_start(wp_f[:], wproj_d.ap())
        wqk = const.tile([128, 512], dt.bfloat16, tag="wqk", name="wqk")
        nc.sync.dma_start(wqk[:], wqk_d.ap())
        wqh, wql = wqk[:, 0:128], wqk[:, 128:256]
        wkh, wkl = wqk[:, 256:384], wqk[:, 384:512]
        wo_r = const.tile([128, 128], dt.float32r, tag="wo_r", name="wo_r")
        nc.vector.tensor_copy(wo_r[:], wp_f[:, 384:512])
        wq_f = wp_f[:, 0:128]
        wk_f = wp_f[:, 128:256]
        wv_f = wp_f[:, 256:384]

        vecs = const.tile([128, 5], dt.float32, tag="vecs", name="vecs")
        nc.sync.dma_start(vecs[:], vecs_d.ap())
        ag_ap, a2_ap, beta_ap = vecs[:, 0:1], vecs[:, 1:2], vecs[:, 2:3]
        wqsum_ap, wksum_ap = vecs[:, 3:4], vecs[:, 4:5]
        rows = const.tile([128, 144], dt.float32, tag="rows", name="rows")
        nc.sync.dma_start(rows[:], rows_d.ap())
        rows4 = const.tile([128, 512], dt.float32, tag="rows4", name="rows4")
        nc.sync.dma_start(rows4[:], rows4_d.ap())

        ident = const.tile([128, 128], dt.bfloat16, tag="ident", name="ident")
        make_identity(nc, ident[:])

        # ---------- conv + stats ----------
        ssum = stat_pool.tile([128, BPC * 5], dt.float32, tag="ssum", name="ssum")
        ssq = stat_pool.tile([128, BPC * 5], dt.float32, tag="ssq", name="ssq")
        y_sb = [ysb_pool.tile([128, LOUT], dt.float32, tag=f"y{b}", name=f"y{b}")
                for b in range(BPC)]

        XW = 520  # 512 + 4 halo each phase (shifts -2..+2)
        for b in range(BPC):
            for lt, (l0, nout) in enumerate(CONV_TILES):
                # xts[(hl, ci, ph)] tile covers phase indices [l0-2, l0-2+XW)
                xts = {}
                for ci in range(2):
                    for hl, src_d in (("h", xh_d), ("l", xl_d)):
                        for ph in range(2):
                            xf = xf_pool.tile(
                                [128, XW], dt.bfloat16,
                                tag=f"xf{hl}{ci}{ph}", name=f"xf{hl}{ci}{ph}")
                            t_lo = l0 - 2
                            v_lo = max(0, t_lo)
                            v_hi = min(LOUT, t_lo + XW)
                            if v_lo > t_lo:
                                nc.gpsimd.memset(xf[:, 0:v_lo - t_lo], 0.0)
                            if v_hi < t_lo + XW:
                                nc.gpsimd.memset(xf[:, v_hi - t_lo:XW], 0.0)
                            nc.sync.dma_start(
                                xf[:, v_lo - t_lo:v_hi - t_lo],
                                src_d.ap()[b, ci * 128:(ci + 1) * 128, ph,
                                           v_lo:v_hi])
                            xts[(hl, ci, ph)] = xf

                ps = bigps.tile([128, 512], dt.float32, tag="b", name="cps")
                first = True
                for ci in range(2):
                    for k in range(KW):
                        j = k * 2 + ci
                        ph = k % 2
                        # tap k reads phase (k%2) at shift s:
                        # even k: s=(k-4)//2; odd k: s=(k-5)//2
                        s = (k - 4) // 2 if ph == 0 else (k - 5) // 2
                        c0 = s + 2  # offset into tile (base l0-2)
                        for hl in ("h", "l"):
                            nc.tensor.matmul(
                                ps[:, 0:nout],
                                wc_bf[:, j * 128:(j + 1) * 128],
                                xts[(hl, ci, ph)][:, c0:c0 + nout],
                                start=first,
                                stop=(hl == "l" and ci == 1 and k == KW - 1))
                            first = False
                col = b * 5 + lt
                nc.scalar.activation(y_sb[b][:, l0:l0 + nout], ps[:, 0:nout],
                                     AF.Copy, accum_out=ssum[:, col:col + 1])
                sq_scr = sqs_pool.tile([128, 512], dt.float32, tag="sqs", name="sqs")
                nc.scalar.activation(sq_scr[:, 0:nout], ps[:, 0:nout],
                                     AF.Square, accum_out=ssq[:, col:col + 1])

        if dbg_d is not None:
            nc.sync.dma_start(dbg_d["conv0"].ap(), y_sb[0][:])

        # ---------- BN stats AllReduce ----------
        ar_sb = bn_pool.tile([128, 2], dt.float32, tag="ar_sb", name="ar_sb")
        nc.vector.reduce_sum(ar_sb[:, 0:1], ssum[:], axis=mybir.AxisListType.X)
        nc.vector.reduce_sum(ar_sb[:, 1:2], ssq[:], axis=mybir.AxisListType.X)
        ar_in = dram.tile([128, 2], dt.float32, tag="ar_in", name="ar_in")
        ar_out = dram.tile([N_CORES * 128, 2], dt.float32, tag="ar_out",
                           name="ar_out")
        nc.sync.dma_start(ar_in[:], ar_sb[:])
        nc.gpsimd.collective_compute(
            "AllGather", ALU.bypass,
            replica_groups=[list(range(N_CORES))],
            ins=[ar_in.opt()], outs=[ar_out.opt()])
        # gather all cores' partials as (128, 2, 8) then reduce over cores
        ar_all = bn_pool.tile([128, 16], dt.float32, tag="ar_all",
                              name="ar_all")
        nc.sync.dma_start(
            ar_all[:].rearrange("p (c r) -> p c r", c=2),
            ar_out[:].rearrange("(r p) c -> p c r", p=128))
        ar_res = bn_pool.tile([128, 2], dt.float32, tag="ar_res", name="ar_res")
        nc.vector.reduce_sum(
            ar_res[:],
            ar_all[:].rearrange("p (c r) -> p c r", c=2),
            axis=mybir.AxisListType.X)

        # keep the PE warm through the collective barrier: a chain of
        # dummy matmuls (WAW on one PSUM tile) gated on the stats reduce
        dmy = smps.tile([128, 128], dt.float32, tag="sm", name="dmy")
        nc.tensor.matmul(dmy[0:1, 0:2], vecs[:, 0:1], ar_sb[:],
                         start=True, stop=True)
        for _ in range(350):
            nc.tensor.matmul(dmy[:], ident[:], ident[:],
                             start=True, stop=True)

        # BN affine: scale = alpha*gamma*rstd, shift = beta - mean*scale
        bnv = bn_pool.tile([128, 8], dt.float32, tag="bnv", name="bnv")
        m_ap = bnv[:, 0:1]
        nc.vector.tensor_scalar(m_ap, ar_res[:, 0:1], 1.0 / NPOS, None, ALU.mult)
        e2_ap = bnv[:, 1:2]
        nc.vector.tensor_scalar(e2_ap, ar_res[:, 1:2], 1.0 / NPOS, None, ALU.mult)
        msq = bnv[:, 2:3]
        nc.vector.tensor_tensor(msq, m_ap, m_ap, ALU.mult)
        var = bnv[:, 3:4]
        nc.vector.tensor_tensor(var, e2_ap, msq, ALU.subtract)
        vy = bnv[:, 4:5]
        nc.vector.tensor_tensor(vy, var, a2_ap, ALU.mult)
        nc.vector.tensor_scalar(vy, vy, EPS, None, ALU.add)
        sd = bnv[:, 5:6]
        nc.scalar.activation(sd, vy, AF.Sqrt)
        rstd = bnv[:, 6:7]
        nc.vector.reciprocal(rstd, sd)

        bnf = bn_pool.tile([128, 4], dt.float32, tag="bnf", name="bnf")
        scale_ap = bnf[:, 0:1]
        nc.vector.tensor_tensor(scale_ap, ag_ap, rstd, ALU.mult)
        shift_ap = bnf[:, 1:2]
        nc.vector.tensor_tensor(shift_ap, m_ap, scale_ap, ALU.mult)
        nc.vector.tensor_tensor(shift_ap, beta_ap, shift_ap, ALU.subtract)
        nscale_ap = bnf[:, 2:3]
        nc.vector.tensor_scalar(nscale_ap, scale_ap, -1.0, None, ALU.mult)
        nshift_ap = bnf[:, 3:4]
        nc.vector.tensor_scalar(nshift_ap, shift_ap, -1.0, None, ALU.mult)

        if dbg_d is not None:
            nc.sync.dma_start(dbg_d["bn"].ap(), bnf[:])

        # ---------- per-sample BN + ELU + attention ----------
        for b in range(BPC):
            s_q = spk_pool.tile([128, LPAD], dt.bfloat16, tag="s_q", name="s_q")
            s_k = spk_pool.tile([128, LPAD], dt.bfloat16, tag="s_k", name="s_k")
            nc.gpsimd.memset(s_q[:, LOUT:LPAD], 0.0)
            nc.gpsimd.memset(s_k[:, LOUT:LPAD], 0.0)
            s_v64 = svp_pool.tile([128, LPAD], dt.bfloat16, tag="s_v64",
                                  name="s_v64")
            nc.gpsimd.memset(s_v64[64:128, 19 * 128:LPAD], 0.0)
            y_hi = hr_pool.tile([128, LOUT], dt.bfloat16, tag="y_hi",
                                name="y_hi")
            y_lo = hr_pool.tile([128, LOUT], dt.bfloat16, tag="y_lo",
                                name="y_lo")

            # chunked BN+ELU fused with projections.
            # y_n = y*scale+shift (GPSIMD); r=max(y_n,0); n=min(y_n,0);
            # y_sb <- r + exp(n) = h+1. spikes use shifted thresholds.
            for lt in range(5):
                l0 = lt * 512
                n = min(512, LOUT - l0)
                sl = slice(l0, l0 + n)
                yn_t = tmp_pool.tile([128, 512], dt.float32, tag="yn_t",
                                     name="yn_t")
                nc.gpsimd.tensor_scalar(yn_t[:, 0:n], y_sb[b][:, sl],
                                        scale_ap, shift_ap, ALU.mult, ALU.add)
                n_t = tmp_pool.tile([128, 512], dt.float32, tag="n_t",
                                    name="n_t")
                nc.vector.tensor_scalar(y_sb[b][:, sl], yn_t[:, 0:n],
                                        0.0, None, ALU.max)
                nc.vector.tensor_scalar(n_t[:, 0:n], yn_t[:, 0:n],
                                        0.0, None, ALU.min)
                nc.scalar.activation(n_t[:, 0:n], n_t[:, 0:n], AF.Exp)
                nc.vector.tensor_tensor(y_sb[b][:, sl], y_sb[b][:, sl],
                                        n_t[:, 0:n], ALU.add)
                nc.gpsimd.tensor_copy(y_hi[:, sl], y_sb[b][:, sl])
                nc.vector.tensor_tensor(y_lo[:, sl], y_sb[b][:, sl],
                                        y_hi[:, sl], ALU.subtract)

                qp = bigps.tile([128, 512], dt.float32, tag="b", name="qps")
                nc.tensor.matmul(qp[:, 0:n], wqh, y_hi[:, sl],
                                 start=True, stop=False)
                nc.tensor.matmul(qp[:, 0:n], wqh, y_lo[:, sl],
                                 start=False, stop=False)
                nc.tensor.matmul(qp[:, 0:n], wql, y_hi[:, sl],
                                 start=False, stop=True)
                nc.vector.tensor_scalar(s_q[:, sl], qp[:, 0:n],
                                        wqsum_ap, None, ALU.is_gt)
                kp = bigps.tile([128, 512], dt.float32, tag="b", name="kps")
                nc.tensor.matmul(kp[:, 0:n], wkh, y_hi[:, sl],
                                 start=True, stop=False)
                nc.tensor.matmul(kp[:, 0:n], wkh, y_lo[:, sl],
                                 start=False, stop=False)
                nc.tensor.matmul(kp[:, 0:n], wkl, y_hi[:, sl],
                                 start=False, stop=True)
                nc.vector.tensor_scalar(s_k[:, sl], kp[:, 0:n],
                                        wksum_ap, None, ALU.is_gt)
                pvk = bigps.tile([128, 512], dt.float32, tag="b",
                                 name="pvk")
                nb = 0
                for t in range(4 * lt, min(4 * lt + 4, POS_TILES)):
                    p0 = t * 128
                    m = min(128, LOUT - p0)
                    if m <= 0:
                        break
                    blk = (t - 4 * lt) * 128
                    nc.tensor.matmul(pvk[0:m, blk:blk + 128],
                                     y_sb[b][:, p0:p0 + m], wv_f,
                                     start=True, stop=True)
                    nb += 1
                if lt < 4:
                    nc.vector.tensor_tensor(
                        s_v64[:, 4 * lt * 128:(4 * lt + nb) * 128],
                        pvk[:, 0:nb * 128], rows4[:, 0:nb * 128], ALU.is_gt)
                else:
                    # tiles 16-18 full; tile 19 valid only for 68 positions
                    nc.vector.tensor_tensor(
                        s_v64[:, 4 * lt * 128:(4 * lt + 3) * 128],
                        pvk[:, 0:384], rows4[:, 0:384], ALU.is_gt)
                    nc.vector.tensor_tensor(
                        s_v64[0:68, 19 * 128:LPAD],
                        pvk[0:68, 384:512], rows4[0:68, 0:128], ALU.is_gt)

            # odd 64-windows need their V spikes at partition base 0:
            sv64t = svp_pool.tile([64, POS_TILES * 128], dt.bfloat16,
                                  tag="sv64t", name="sv64t")
            for t in range(POS_TILES):
                nc.sync.dma_start(sv64t[0:64, t * 128:(t + 1) * 128],
                                  s_v64[64:128, t * 128:(t + 1) * 128])

            if dbg_d is not None and b == 0:
                nc.sync.dma_start(dbg_d["h0"].ap(), y_sb[b][:])
                nc.sync.dma_start(dbg_d["sq0"].ap(), s_q[:])
                nc.sync.dma_start(dbg_d["sk0"].ap(), s_k[:])
                nc.sync.dma_start(dbg_d["sv0"].ap(), s_v64[:])

            # ---- global attention: pool y per chunk, subtract counts ----
            hsum = pool_pool.tile([128, GPOOL], dt.float32, tag="hsum",
                                  name="hsum")
            for g in range(GPOOL):
                c0 = CHUNK * g
                c1 = min(CHUNK * (g + 1), LOUT)
                nc.vector.reduce_sum(hsum[:, g:g + 1], y_sb[b][:, c0:c1],
                                     axis=mybir.AxisListType.X)
            nc.vector.tensor_tensor(hsum[:], hsum[:],
                                    rows[:, 128:128 + GPOOL], ALU.subtract)
            kgp = smps.tile([128, 128], dt.float32, tag="sm", name="kgp")
            nc.tensor.matmul(kgp[:, 0:GPOOL], wk_f, hsum[:],
                             start=True, stop=True)
            vgp = smps.tile([128, 128], dt.float32, tag="sm", name="vgp")
            nc.tensor.matmul(vgp[:, 0:GPOOL], wv_f, hsum[:],
                             start=True, stop=True)
            sg = pool_pool.tile([128, 2 * GPOOL], dt.bfloat16, tag="sg",
                                name="sg")
            nc.vector.tensor_scalar(sg[:, 0:GPOOL], kgp[:, 0:GPOOL],
                                    0.0, None, ALU.is_gt)
            nc.vector.tensor_scalar(sg[:, GPOOL:2 * GPOOL], vgp[:, 0:GPOOL],
                                    0.0, None, ALU.is_gt)
            tp_k = smps.tile([16, 128], dt.bfloat16, tag="sm", name="tp_k")
            nc.tensor.transpose(tp_k[:], sg[:, 0:GPOOL], ident[:])
            tp_v = smps.tile([16, 128], dt.bfloat16, tag="sm", name="tp_v")
            nc.tensor.transpose(tp_v[:], sg[:, GPOOL:2 * GPOOL], ident[:])
            sgt = pool_pool.tile([16, 256], dt.bfloat16, tag="sgt", name="sgt")
            nc.vector.tensor_copy(sgt[:, 0:128], tp_k[:])
            nc.vector.tensor_copy(sgt[:, 128:256], tp_v[:])
            kvp = smps.tile([128, 128], dt.float32, tag="sm", name="kvp")
            nc.tensor.matmul(kvp[:], sgt[:, 0:128], sgt[:, 128:256],
                             start=True, stop=True)
            kv_bf = pool_pool.tile([128, 128], dt.bfloat16, tag="kv_bf",
                                   name="kv_bf")
            nc.vector.tensor_scalar(kv_bf[:], kvp[:], 1.0 / GPOOL, None,
                                    ALU.mult)

            # ---- attention l-tiles + fused wo/residual/output ----
            for lt in range(ATT_LT):
                l0 = lt * 512
                ap_t = bigps.tile([128, 512], dt.float32, tag="b", name="attps")
                # attnT blocks first (both scales), evacs overlap out-matmuls
                a64pk = a256ps.tile([64, 512], dt.float32, tag="a64pk",
                                    name="a64pk")
                for nwin in range(8):
                    w0 = l0 + nwin * 64
                    nc.tensor.matmul(a64pk[:, nwin * 64:nwin * 64 + 64],
                                     s_k[:, w0:w0 + 64],
                                     s_q[:, w0:w0 + 64], start=True, stop=True)
                a64b = abf_pool.tile([64, 512], dt.bfloat16, tag="a64b",
                                     name="a64b")
                nc.scalar.activation(a64b[:], a64pk[:], AF.Copy,
                                     scale=1.0 / 64)
                a256pk = a256ps.tile([128, 1024], dt.float32, tag="a256pk",
                                     name="a256pk")
                for mwin in range(2):
                    w0 = l0 + mwin * 256
                    for uh in range(2):
                        blk = (mwin * 2 + uh) * 256
                        nc.tensor.matmul(
                            a256pk[:, blk:blk + 256],
                            s_k[:, w0 + uh * 128:w0 + uh * 128 + 128],
                            s_q[:, w0:w0 + 256],
                            start=True, stop=True)
                a256b = abf_pool.tile([128, 1024], dt.bfloat16, tag="a256b",
                                      name="a256b")
                nc.scalar.activation(a256b[:], a256pk[:], AF.Copy,
                                     scale=1.0 / 256)
                # global fills the whole tile (start=True clears the bank),
                # overlapping the ACT evacuations of the attnT blocks
                nc.tensor.matmul(ap_t[:], kv_bf[:], s_q[:, l0:l0 + 512],
                                 start=True, stop=False)
                for nwin in range(8):
                    w0 = l0 + nwin * 64
                    t = w0 // 128
                    if w0 % 128 == 0:
                        v_lhsT = s_v64[0:64, t * 128:(t + 1) * 128]
                    else:
                        v_lhsT = sv64t[0:64, t * 128:(t + 1) * 128]
                    nc.tensor.matmul(
                        ap_t[:, nwin * 64:nwin * 64 + 64],
                        v_lhsT, a64b[:, nwin * 64:nwin * 64 + 64],
                        start=False, stop=False)
                for mwin in range(2):
                    w0 = l0 + mwin * 256
                    for uh in range(2):
                        blk = (mwin * 2 + uh) * 256
                        t = (w0 + uh * 128) // 128
                        last = (mwin == 1 and uh == 1)
                        nc.tensor.matmul(
                            ap_t[:, mwin * 256:mwin * 256 + 256],
                            s_v64[:, t * 128:(t + 1) * 128],
                            a256b[:, blk:blk + 256], start=False, stop=last)
                o2r = o2_pool.tile([128, 512], dt.float32r, tag="o2r",
                                   name="o2r")
                nc.vector.tensor_copy(o2r[:], ap_t[:])
                if dbg_d is not None and b == 0:
                    nc.sync.dma_start(
                        dbg_d["o20"].ap()[:, l0:l0 + 512],
                        o2r[:].bitcast(dt.float32))

                # wo projection + residual + output for this tile
                n = min(512, LOUT - l0)
                fp = bigps.tile([128, 512], dt.float32, tag="b", name="fps")
                nc.tensor.matmul(fp[:, 0:n], wo_r, o2r[:, 0:n],
                                 start=True, stop=True)
                fin = fin_pool.tile([128, 512], dt.float32, tag="fin",
                                    name="fin")
                nc.scalar.activation(fin[:, 0:n], fp[:, 0:n], AF.Copy,
                                     bias=-1.0)
                nc.gpsimd.tensor_tensor(fin[:, 0:n], fin[:, 0:n],
                                        y_sb[b][:, l0:l0 + n], ALU.add)
                nc.sync.dma_start(yout_d.ap()[b, :, l0:l0 + n], fin[:, 0:n])


_NC_CACHE = {}


def _get_nc():
    if "nc" not in _NC_CACHE:
        _NC_CACHE["nc"] = _build_kernel()
    return _NC_CACHE["nc"]


def make_in_maps(x, conv_w, conv_b, gamma, beta, wq, wk, wv, wo):
    import ml_dtypes
    bf16 = ml_dtypes.bfloat16
    x = np.asarray(x, dtype=np.float32)
    conv_w = np.asarray(conv_w, dtype=np.float32)
    gamma = np.asarray(gamma, dtype=np.float32)
    beta = np.asarray(beta, dtype=np.float32)

    x_hi = x.astype(bf16)
    x_lo = (x - x_hi.astype(np.float32)).astype(bf16)
    # deinterleave stride-2 phases: (B, CIN, 2, 2500), phase p holds x[..., 2t+p]
    x_hi = np.ascontiguousarray(
        x_hi.reshape(B, CIN, LOUT, 2).transpose(0, 1, 3, 2))
    x_lo = np.ascontiguousarray(
        x_lo.reshape(B, CIN, LOUT, 2).transpose(0, 1, 3, 2))

    # host-side weight prep
    sign_w = np.sign(conv_w).astype(np.float32)            # (COUT, CIN, KW)
    alpha = np.abs(conv_w).mean(axis=(1, 2)).astype(np.float32)
    wconv = np.ascontiguousarray(
        sign_w.transpose(2, 1, 0).reshape(KW, 2, 128, COUT)).astype(bf16)
    wq = np.asarray(wq, dtype=np.float32)
    wk = np.asarray(wk, dtype=np.float32)
    wv = np.asarray(wv, dtype=np.float32)
    wproj = np.concatenate(
        [wq, wk, wv, np.asarray(wo) / 3.0], axis=1).astype(np.float32)
    vecs = np.stack([alpha * gamma, alpha * alpha, beta,
                     wq.sum(axis=0), wk.sum(axis=0)],
                    axis=1).astype(np.float32)              # (128, 5)
    wqk_cat = np.concatenate([wq, wk], axis=1)          # (128, 256)
    wqk_hi = wqk_cat.astype(bf16)
    wqk_lo = (wqk_cat - wqk_hi.astype(np.float32)).astype(bf16)
    wqk_hl = np.concatenate(
        [wqk_hi[:, 0:128], wqk_lo[:, 0:128],
         wqk_hi[:, 128:256], wqk_lo[:, 128:256]], axis=1)  # (128, 512)
    cnt = np.full(GPOOL, float(CHUNK), np.float32)
    cnt[-1] = LOUT - CHUNK * (GPOOL - 1)
    rows = np.tile(np.concatenate([wv.sum(axis=0), cnt]), (128, 1)).astype(
        np.float32)
    rows4 = np.tile(wv.sum(axis=0), (128, 4)).astype(np.float32)

    in_maps = []
    for c in range(N_CORES):
        in_maps.append({
            "xs_hi": np.ascontiguousarray(x_hi[c * BPC:(c + 1) * BPC]),
            "xs_lo": np.ascontiguousarray(x_lo[c * BPC:(c + 1) * BPC]),
            "wconv": wconv,
            "wproj": wproj,
            "wqk_hl": wqk_hl,
            "vecs": vecs,
            "rows": rows,
            "rows4": rows4,
        })
    return in_maps


def kernel(x, conv_w, conv_b, gamma, beta, wq, wk, wv, wo):
    in_maps = make_in_maps(x, conv_w, conv_b, gamma, beta, wq, wk, wv, wo)
    nc = _get_nc()
    res = run_bass_kernel_spmd(nc, in_maps, core_ids=list(range(N_CORES)))
    out = np.concatenate([res.results[c]["yout"] for c in range(N_CORES)],
                         axis=0)
    return out.astype(np.float32)



# revision 2
# speedup vs baseline: 1.2499x; 1.2499x over previous
"""Trainium2 Bass kernel for BeatPatternExtractor, v2.

quantized conv1d (stride 2) -> training-mode BatchNorm -> ELU -> multi-scale
spiking window/global attention with residual.

Sharding: data-parallel over batch (32 samples -> 4 per core x 8 cores).

v2 changes vs baseline:
- conv: x split into fp8 e4m3 hi+lo, DoubleRow-paired over the two
  128-channel halves -> 9 K=256 matmuls per (pass, tile) at 2x fp8 rate.
  Two passes (hi, lo); BN stats come from the hi pass only so the stats
  AllReduce overlaps the lo pass (no dummy matmuls, no idle hole).
- projections: Q/K single fp32r matmul each (full rate at N>=512, FP22
  mantissa); V via bf16 h; wo via fp32r.
- ELU: h+1 = Relu(scale*y+shift) + min(Exp(scale*y+shift), 1) -- affine
  folded into two ACT ops, no GPSIMD cast chain.
- 64-window attention: block-diagonal pairing -> 4 N=128 qk matmuls and
  4 N=128 out matmuls per 512-tile (was 8+8 N=64), no sv64t copies.
- residual h added via identity matmul into the wo PSUM accumulation.
"""
import sys

sys.path.insert(0, "/opt/trn_rl_repo")

import numpy as np

import concourse.bass as bass  # noqa: F401
import concourse.mybir as mybir
import concourse.tile as tile
from concourse import bacc
from concourse.bass_utils import run_bass_kernel_spmd
from concourse.masks import make_identity

dt = mybir.dt
AF = mybir.ActivationFunctionType
ALU = mybir.AluOpType
DR = mybir.MatmulPerfMode.DoubleRow

N_CORES = 8
B, CIN, L = 32, 256, 5000
COUT, KW = 128, 9
LOUT = 2500
BPC = B // N_CORES
LPAD = 2560
NPOS = B * LOUT
CHUNK = 157
GPOOL = 16
EPS = 1e-5
XW = 528                       # pair stride must be %16==0; covers 520 cols

CONV_TILES = [(0, 512), (512, 512), (1024, 512), (1536, 512), (2048, 452)]
ATT_LT = 5
POS_TILES = 20


def _build_kernel():
    nc = bacc.Bacc("TRN2", target_bir_lowering=False, debug=False,
                   num_devices=N_CORES)

    x8_d = nc.dram_tensor("x8", [BPC, 2, 2, 128, 2, LOUT], dt.float8e4,
                          kind="ExternalInput")
    wconv_d = nc.dram_tensor("wconv8", [KW, 128, 256], dt.float8e4,
                             kind="ExternalInput")
    wproj_d = nc.dram_tensor("wproj", [128, 4 * 128], dt.float32r,
                             kind="ExternalInput")
    vecs_d = nc.dram_tensor("vecs", [128, 5], dt.float32, kind="ExternalInput")
    rows_d = nc.dram_tensor("rows", [128, 144], dt.float32, kind="ExternalInput")
    rows4_d = nc.dram_tensor("rows4", [128, 512], dt.float32,
                             kind="ExternalInput")
    yout_d = nc.dram_tensor("yout", [BPC, COUT, LOUT], dt.float32,
                            kind="ExternalOutput")

    with tile.TileContext(nc) as tc:
        _body(tc, nc, x8_d, wconv_d, wproj_d, vecs_d, rows_d, rows4_d, yout_d)
    nc.compile()
    return nc


def _conv_pass(nc, xf_pool, bigps, x8_d, wc8, b, lt, hl, evac):
    """One conv pass (hi or lo) for sample b, tile lt; evac(ps, l0, n)."""
    l0, nout = CONV_TILES[lt]
    t_lo = l0 - 2
    xfs = []
    for ph in range(2):
        xf = xf_pool.tile([128, 2, XW], dt.float8e4, tag=f"xf{hl}{ph}",
                          name=f"xf{hl}{ph}")
        v_lo = max(0, t_lo)
        v_hi = min(LOUT, t_lo + 520)
        if v_lo > t_lo:
            nc.gpsimd.memset(xf[:, :, 0:v_lo - t_lo], 0.0)
        if v_hi < t_lo + 520:
            nc.gpsimd.memset(xf[:, :, v_hi - t_lo:XW], 0.0)
        nc.sync.dma_start(xf[:, :, v_lo - t_lo:v_hi - t_lo],
                          x8_d.ap()[b, hl, ph, :, :, v_lo:v_hi])
        xfs.append(xf)
    ps = bigps.tile([128, 512], dt.float32, tag="b", name=f"cps{hl}")
    for k in range(KW):
        ph = k % 2
        s = (k - 4) // 2 if ph == 0 else (k - 5) // 2
        c0 = s + 2
        lhsT = wc8[:, k * 256:(k + 1) * 256].rearrange("p (c o) -> p c o", c=2)
        nc.tensor.matmul(ps[:, 0:nout], lhsT, xfs[ph][:, :, c0:c0 + nout],
                         start=(k == 0), stop=(k == KW - 1), perf_mode=DR)
    evac(ps, l0, nout)


def _body(tc, nc, x8_d, wconv_d, wproj_d, vecs_d, rows_d, rows4_d, yout_d):
    import contextlib
    ctx = contextlib.ExitStack()
    with ctx:
        const = ctx.enter_context(tc.tile_pool(name="const", bufs=1))
        xf_pool = ctx.enter_context(tc.tile_pool(name="xf", bufs=3))
        ysb_pool = ctx.enter_context(tc.tile_pool(name="ysb", bufs=1))
        stat_pool = ctx.enter_context(tc.tile_pool(name="stat", bufs=1))
        sqs_pool = ctx.enter_context(tc.tile_pool(name="sqs", bufs=2))
        bn_pool = ctx.enter_context(tc.tile_pool(name="bn", bufs=1))
        hr_pool = ctx.enter_context(tc.tile_pool(name="hr", bufs=2))
        spk_pool = ctx.enter_context(tc.tile_pool(name="spk", bufs=2))
        svp_pool = ctx.enter_context(tc.tile_pool(name="svp", bufs=2))
        pool_pool = ctx.enter_context(tc.tile_pool(name="pool", bufs=2))
        abf_pool = ctx.enter_context(tc.tile_pool(name="abf", bufs=2))
        o2_pool = ctx.enter_context(tc.tile_pool(name="o2", bufs=3))
        fin_pool = ctx.enter_context(tc.tile_pool(name="fin", bufs=2))
        tmp_pool = ctx.enter_context(tc.tile_pool(name="tmp", bufs=3))

        bigps = ctx.enter_context(tc.tile_pool(name="bigps", bufs=3,
                                               space="PSUM"))
        a256ps = ctx.enter_context(tc.tile_pool(name="a256ps", bufs=1,
                                                space="PSUM"))
        smps = ctx.enter_context(tc.tile_pool(name="smps", bufs=2,
                                              space="PSUM"))

        dram = ctx.enter_context(tc.tile_pool(name="dram", bufs=1,
                                              space="DRAM"))

        # ---------- constants / weights ----------
        wc8 = const.tile([128, KW * 256], dt.float8e4, tag="wc8", name="wc8")
        for k in range(KW):
            nc.sync.dma_start(wc8[:, k * 256:(k + 1) * 256], wconv_d.ap()[k])

        wp_f = const.tile([128, 512], dt.float32r, tag="wp_f", name="wp_f")
        nc.sync.dma_start(wp_f[:], wproj_d.ap())
        wq_r = wp_f[:, 0:128]
        wk_r = wp_f[:, 128:256]
        wv_r = wp_f[:, 256:384]
        wo_r = wp_f[:, 384:512]
        wv_b = const.tile([128, 128], dt.bfloat16, tag="wv_b", name="wv_b")
        nc.vector.tensor_copy(wv_b[:], wv_r)

        vecs = const.tile([128, 5], dt.float32, tag="vecs", name="vecs")
        nc.sync.dma_start(vecs[:], vecs_d.ap())
        ag_ap, a2_ap, beta_ap = vecs[:, 0:1], vecs[:, 1:2], vecs[:, 2:3]
        wqsum_ap, wksum_ap = vecs[:, 3:4], vecs[:, 4:5]
        rows = const.tile([128, 144], dt.float32, tag="rows", name="rows")
        nc.sync.dma_start(rows[:], rows_d.ap())
        rows4 = const.tile([128, 512], dt.float32, tag="rows4", name="rows4")
        nc.sync.dma_start(rows4[:], rows4_d.ap())

        ident = const.tile([128, 128], dt.bfloat16, tag="ident", name="ident")
        make_identity(nc, ident[:])

        # persistent zero-padded diag buffers for 64-window attn
        a64d = [const.tile([128, 512], dt.bfloat16, tag=f"a64d{i}",
                           name=f"a64d{i}") for i in range(2)]
        for t in a64d:
            nc.gpsimd.memset(t[:], 0.0)

        # ---------- conv hi pass + stats ----------
        ssum = stat_pool.tile([128, BPC * 5], dt.float32, tag="ssum",
                              name="ssum")
        ssq = stat_pool.tile([128, BPC * 5], dt.float32, tag="ssq", name="ssq")
        y_sb = [ysb_pool.tile([128, LOUT], dt.float32, tag=f"y{b}",
                              name=f"y{b}") for b in range(BPC)]

        for b in range(BPC):
            for lt in range(5):
                def evac_hi(ps, l0, n, b=b, lt=lt):
                    col = b * 5 + lt
                    nc.scalar.activation(y_sb[b][:, l0:l0 + n], ps[:, 0:n],
                                         AF.Copy,
                                         accum_out=ssum[:, col:col + 1])
                    sq_scr = sqs_pool.tile([128, 512], dt.float32, tag="sqs",
                                           name="sqs")
                    nc.scalar.activation(sq_scr[:, 0:n], ps[:, 0:n],
                                         AF.Square,
                                         accum_out=ssq[:, col:col + 1])
                _conv_pass(nc, xf_pool, bigps, x8_d, wc8, b, lt, 0, evac_hi)

        # ---------- BN stats AllReduce (overlaps conv lo pass) ----------
        ar_sb = bn_pool.tile([128, 2], dt.float32, tag="ar_sb", name="ar_sb")
        nc.vector.reduce_sum(ar_sb[:, 0:1], ssum[:], axis=mybir.AxisListType.X)
        nc.vector.reduce_sum(ar_sb[:, 1:2], ssq[:], axis=mybir.AxisListType.X)
        ar_in = dram.tile([128, 2], dt.float32, tag="ar_in", name="ar_in")
        ar_out = dram.tile([N_CORES * 128, 2], dt.float32, tag="ar_out",
                           name="ar_out")
        nc.sync.dma_start(ar_in[:], ar_sb[:])
        nc.gpsimd.collective_compute(
            "AllGather", ALU.bypass,
            replica_groups=[list(range(N_CORES))],
            ins=[ar_in.opt()], outs=[ar_out.opt()])

        # ---------- conv lo pass (runs during AllGather) ----------
        for b in range(BPC):
            for lt in range(5):
                def evac_lo(ps, l0, n, b=b):
                    nc.vector.tensor_tensor(y_sb[b][:, l0:l0 + n],
                                            y_sb[b][:, l0:l0 + n],
                                            ps[:, 0:n], ALU.add)
                _conv_pass(nc, xf_pool, bigps, x8_d, wc8, b, lt, 1, evac_lo)

        # ---------- combine stats, BN affine ----------
        ar_all = bn_pool.tile([128, 16], dt.float32, tag="ar_all",
                              name="ar_all")
        nc.sync.dma_start(
            ar_all[:].rearrange("p (c r) -> p c r", c=2),
            ar_out[:].rearrange("(r p) c -> p c r", p=128))
        ar_res = bn_pool.tile([128, 2], dt.float32, tag="ar_res", name="ar_res")
        nc.vector.reduce_sum(
            ar_res[:],
            ar_all[:].rearrange("p (c r) -> p c r", c=2),
            axis=mybir.AxisListType.X)

        bnv = bn_pool.tile([128, 8], dt.float32, tag="bnv", name="bnv")
        m_ap = bnv[:, 0:1]
        nc.vector.tensor_scalar(m_ap, ar_res[:, 0:1], 1.0 / NPOS, None,
                                ALU.mult)
        e2_ap = bnv[:, 1:2]
        nc.vector.tensor_scalar(e2_ap, ar_res[:, 1:2], 1.0 / NPOS, None,
                                ALU.mult)
        msq = bnv[:, 2:3]
        nc.vector.tensor_tensor(msq, m_ap, m_ap, ALU.mult)
        var = bnv[:, 3:4]
        nc.vector.tensor_tensor(var, e2_ap, msq, ALU.subtract)
        vy = bnv[:, 4:5]
        nc.vector.tensor_tensor(vy, var, a2_ap, ALU.mult)
        nc.vector.tensor_scalar(vy, vy, EPS, None, ALU.add)
        sd = bnv[:, 5:6]
        nc.scalar.activation(sd, vy, AF.Sqrt)
        rstd = bnv[:, 6:7]
        nc.vector.reciprocal(rstd, sd)

        bnf = bn_pool.tile([128, 4], dt.float32, tag="bnf", name="bnf")
        scale_ap = bnf[:, 0:1]
        nc.vector.tensor_tensor(scale_ap, ag_ap, rstd, ALU.mult)
        shift_ap = bnf[:, 1:2]
        nc.vector.tensor_tensor(shift_ap, m_ap, scale_ap, ALU.mult)
        nc.vector.tensor_tensor(shift_ap, beta_ap, shift_ap, ALU.subtract)

        # ---------- per-sample BN + ELU + attention ----------
        for b in range(BPC):
            s_q = spk_pool.tile([128, LPAD], dt.bfloat16, tag="s_q",
                                name="s_q")
            s_k = spk_pool.tile([128, LPAD], dt.bfloat16, tag="s_k",
                                name="s_k")
            nc.gpsimd.memset(s_q[:, LOUT:LPAD], 0.0)
            nc.gpsimd.memset(s_k[:, LOUT:LPAD], 0.0)
            s_v64 = svp_pool.tile([128, LPAD], dt.bfloat16, tag="s_v64",
                                  name="s_v64")
            nc.gpsimd.memset(s_v64[64:128, 19 * 128:LPAD], 0.0)
            h1f = hr_pool.tile([128, LOUT], dt.float32r, tag="h1f", name="h1f")
            h1b = hr_pool.tile([128, LOUT], dt.bfloat16, tag="h1b",
                               name="h1b")

            # BN + ELU + projections, chunked
            for lt in range(5):
                l0 = lt * 512
                n = min(512, LOUT - l0)
                sl = slice(l0, l0 + n)
                ex = tmp_pool.tile([128, 512], dt.float32, tag="ex", name="ex")
                nc.scalar.activation(ex[:, 0:n], y_sb[b][:, sl], AF.Exp,
                                     bias=shift_ap, scale=scale_ap)
                nc.scalar.activation(h1f[:, sl], y_sb[b][:, sl], AF.Relu,
                                     bias=shift_ap, scale=scale_ap)
                nc.vector.tensor_scalar(ex[:, 0:n], ex[:, 0:n], 1.0, None,
                                        ALU.min)
                nc.vector.tensor_tensor(h1f[:, sl], h1f[:, sl], ex[:, 0:n],
                                        ALU.add)
                nc.gpsimd.tensor_copy(h1b[:, sl], h1f[:, sl])

                qp = bigps.tile([128, 512], dt.float32, tag="b", name="qps")
                nc.tensor.matmul(qp[:, 0:n], wq_r, h1f[:, sl],
                                 start=True, stop=True)
                nc.vector.tensor_scalar(s_q[:, sl], qp[:, 0:n],
                                        wqsum_ap, None, ALU.is_gt)
                kp = bigps.tile([128, 512], dt.float32, tag="b", name="kps")
                nc.tensor.matmul(kp[:, 0:n], wk_r, h1f[:, sl],
                                 start=True, stop=True)
                nc.vector.tensor_scalar(s_k[:, sl], kp[:, 0:n],
                                        wksum_ap, None, ALU.is_gt)
                pvk = bigps.tile([128, 512], dt.float32, tag="b", name="pvk")
                nb = 0
                for t in range(4 * lt, min(4 * lt + 4, POS_TILES)):
                    p0 = t * 128
                    m = min(128, LOUT - p0)
                    if m <= 0:
                        break
                    blk = (t - 4 * lt) * 128
                    nc.tensor.matmul(pvk[0:m, blk:blk + 128],
                                     h1b[:, p0:p0 + m], wv_b,
                                     start=True, stop=True)
                    nb += 1
                if lt < 4:
                    nc.vector.tensor_tensor(
                        s_v64[:, 4 * lt * 128:(4 * lt + nb) * 128],
                        pvk[:, 0:nb * 128], rows4[:, 0:nb * 128], ALU.is_gt)
                else:
                    nc.vector.tensor_tensor(
                        s_v64[:, 4 * lt * 128:(4 * lt + 3) * 128],
                        pvk[:, 0:384], rows4[:, 0:384], ALU.is_gt)
                    nc.vector.tensor_tensor(
                        s_v64[0:68, 19 * 128:LPAD],
                        pvk[0:68, 384:512], rows4[0:68, 0:128], ALU.is_gt)

            # ---- global attention ----
            hsum = pool_pool.tile([128, GPOOL], dt.float32r, tag="hsum",
                                  name="hsum")
            with nc.allow_low_precision(reason="fp32r is fp32-width"):
                nc.vector.reduce_sum(
                    hsum[:, 0:15],
                    h1b[:, 0:15 * CHUNK].rearrange("p (g c) -> p g c", g=15),
                    axis=mybir.AxisListType.X)
                nc.vector.reduce_sum(hsum[:, 15:16], h1b[:, 15 * CHUNK:LOUT],
                                     axis=mybir.AxisListType.X)
            nc.vector.tensor_tensor(hsum[:], hsum[:],
                                    rows[:, 128:128 + GPOOL], ALU.subtract)
            kgp = smps.tile([128, 128], dt.float32, tag="sm", name="kgp")
            nc.tensor.matmul(kgp[:, 0:GPOOL], wk_r, hsum[:],
                             start=True, stop=True)
            vgp = smps.tile([128, 128], dt.float32, tag="sm", name="vgp")
            nc.tensor.matmul(vgp[:, 0:GPOOL], wv_r, hsum[:],
                             start=True, stop=True)
            sg = pool_pool.tile([128, 2 * GPOOL], dt.bfloat16, tag="sg",
                                name="sg")
            nc.vector.tensor_scalar(sg[:, 0:GPOOL], kgp[:, 0:GPOOL],
                                    0.0, None, ALU.is_gt)
            nc.vector.tensor_scalar(sg[:, GPOOL:2 * GPOOL], vgp[:, 0:GPOOL],
                                    0.0, None, ALU.is_gt)
            tp_k = smps.tile([16, 128], dt.bfloat16, tag="sm", name="tp_k")
            nc.tensor.transpose(tp_k[:], sg[:, 0:GPOOL], ident[:])
            tp_v = smps.tile([16, 128], dt.bfloat16, tag="sm", name="tp_v")
            nc.tensor.transpose(tp_v[:], sg[:, GPOOL:2 * GPOOL], ident[:])
            sgt = pool_pool.tile([16, 256], dt.bfloat16, tag="sgt",
                                 name="sgt")
            nc.vector.tensor_copy(sgt[:, 0:128], tp_k[:])
            nc.vector.tensor_copy(sgt[:, 128:256], tp_v[:])
            kvp = smps.tile([128, 128], dt.float32, tag="sm", name="kvp")
            nc.tensor.matmul(kvp[:], sgt[:, 0:128], sgt[:, 128:256],
                             start=True, stop=True)
            kv_bf = pool_pool.tile([128, 128], dt.bfloat16, tag="kv_bf",
                                   name="kv_bf")
            nc.vector.tensor_scalar(kv_bf[:], kvp[:], 1.0 / GPOOL, None,
                                    ALU.mult)

            # ---- attention l-tiles + fused wo/residual/output ----
            for lt in range(ATT_LT):
                l0 = lt * 512
                ad = a64d[(b * 5 + lt) % 2]
                a64pk = a256ps.tile([128, 512], dt.float32, tag="a64pk",
                                    name="a64pk")
                for p in range(4):
                    w0 = l0 + p * 128
                    nc.tensor.matmul(a64pk[:, p * 128:(p + 1) * 128],
                                     s_k[:, w0:w0 + 128],
                                     s_q[:, w0:w0 + 128],
                                     start=True, stop=True)
                # diag evac: even windows -> rows 0-63 cols 0-63 of each
                # 128-block; odd windows -> rows 64-127 cols 64-127
                src_e = a64pk[0:64, :].rearrange("p (f c) -> p f c", f=4)
                dst_e = ad[0:64, :].rearrange("p (f c) -> p f c", f=4)
                nc.scalar.activation(dst_e[:, :, 0:64], src_e[:, :, 0:64],
                                     AF.Copy, scale=1.0 / 64)
                src_o = a64pk[64:128, :].rearrange("p (f c) -> p f c", f=4)
                dst_o = ad[64:128, :].rearrange("p (f c) -> p f c", f=4)
                nc.scalar.activation(dst_o[:, :, 64:128], src_o[:, :, 64:128],
                                     AF.Copy, scale=1.0 / 64)

                a256pk = a256ps.tile([128, 1024], dt.float32, tag="a256pk",
                                     name="a256pk")
                for mwin in range(2):
                    w0 = l0 + mwin * 256
                    for uh in range(2):
                        blk = (mwin * 2 + uh) * 256
                        nc.tensor.matmul(
                            a256pk[:, blk:blk + 256],
                            s_k[:, w0 + uh * 128:w0 + uh * 128 + 128],
                            s_q[:, w0:w0 + 256],
                            start=True, stop=True)
                a256b = abf_pool.tile([128, 1024], dt.bfloat16, tag="a256b",
                                      name="a256b")
                nc.scalar.activation(a256b[:], a256pk[:], AF.Copy,
                                     scale=1.0 / 256)

                ap_t = bigps.tile([128, 512], dt.float32, tag="b",
                                  name="attps")
                nc.tensor.matmul(ap_t[:], kv_bf[:], s_q[:, l0:l0 + 512],
                                 start=True, stop=False)
                for p in range(4):
                    t = (l0 + p * 128) // 128
                    nc.tensor.matmul(
                        ap_t[:, p * 128:(p + 1) * 128],
                        s_v64[:, t * 128:(t + 1) * 128],
                        ad[:, p * 128:(p + 1) * 128],
                        start=False, stop=False)
                for mwin in range(2):
                    w0 = l0 + mwin * 256
                    for uh in range(2):
                        blk = (mwin * 2 + uh) * 256
                        t = (w0 + uh * 128) // 128
                        last = (mwin == 1 and uh == 1)
                        nc.tensor.matmul(
                            ap_t[:, mwin * 256:mwin * 256 + 256],
                            s_v64[:, t * 128:(t + 1) * 128],
                            a256b[:, blk:blk + 256], start=False, stop=last)
                o2r = o2_pool.tile([128, 512], dt.float32r, tag="o2r",
                                   name="o2r")
                nc.vector.tensor_copy(o2r[:], ap_t[:])

                n = min(512, LOUT - l0)
                fp = bigps.tile([128, 512], dt.float32, tag="b", name="fps")
                nc.tensor.matmul(fp[:, 0:n], wo_r, o2r[:, 0:n],
                                 start=True, stop=False)
                nc.tensor.matmul(fp[:, 0:n], ident[:], h1b[:, l0:l0 + n],
                                 start=False, stop=True)
                fin = fin_pool.tile([128, 512], dt.float32, tag="fin",
                                    name="fin")
                nc.scalar.activation(fin[:, 0:n], fp[:, 0:n], AF.Copy,
                                     bias=-1.0)
                nc.sync.dma_start(yout_d.ap()[b, :, l0:l0 + n], fin[:, 0:n])


_NC_CACHE = {}


def _get_nc():
    if "nc" not in _NC_CACHE:
        _NC_CACHE["nc"] = _build_kernel()
    return _NC_CACHE["nc"]


def make_in_maps(x, conv_w, conv_b, gamma, beta, wq, wk, wv, wo):
    import ml_dtypes
    bf16 = ml_dtypes.bfloat16
    e4 = ml_dtypes.float8_e4m3
    x = np.asarray(x, dtype=np.float32)
    conv_w = np.asarray(conv_w, dtype=np.float32)
    gamma = np.asarray(gamma, dtype=np.float32)
    beta = np.asarray(beta, dtype=np.float32)

    x_hi = x.astype(e4)
    x_lo = (x - x_hi.astype(np.float32)).astype(e4)
    # [2(hl), B, 2(ci), 128(p), 2500(t), 2(ph)] -> [B, hl, ph, p, ci, t]
    arr = np.stack([x_hi, x_lo])
    arr = arr.reshape(2, B, 2, 128, LOUT, 2)
    x8 = np.ascontiguousarray(arr.transpose(1, 0, 5, 3, 2, 4))

    sign_w = np.sign(conv_w).astype(np.float32)            # (O, CIN, KW)
    alpha = np.abs(conv_w).mean(axis=(1, 2)).astype(np.float32)
    # [KW, 128(p), 2(ci), 128(o)] -> [KW, 128, 256]
    w9 = sign_w.transpose(2, 1, 0).reshape(KW, 2, 128, COUT)
    wconv8 = np.ascontiguousarray(
        w9.transpose(0, 2, 1, 3).reshape(KW, 128, 256)).astype(e4)

    wq = np.asarray(wq, dtype=np.float32)
    wk = np.asarray(wk, dtype=np.float32)
    wv = np.asarray(wv, dtype=np.float32)
    wproj = np.concatenate(
        [wq, wk, wv, np.asarray(wo, dtype=np.float32) / 3.0],
        axis=1).astype(np.float32)
    vecs = np.stack([alpha * gamma, alpha * alpha, beta,
                     wq.sum(axis=0), wk.sum(axis=0)],
                    axis=1).astype(np.float32)
    cnt = np.full(GPOOL, float(CHUNK), np.float32)
    cnt[-1] = LOUT - CHUNK * (GPOOL - 1)
    rows = np.tile(np.concatenate([wv.sum(axis=0), cnt]), (128, 1)).astype(
        np.float32)
    wvb = wv.astype(bf16).astype(np.float32)
    rows4 = np.tile(wvb.sum(axis=0), (128, 4)).astype(np.float32)

    in_maps = []
    for c in range(N_CORES):
        in_maps.append({
            "x8": np.ascontiguousarray(x8[c * BPC:(c + 1) * BPC]),
            "wconv8": wconv8,
            "wproj": wproj,
            "vecs": vecs,
            "rows": rows,
            "rows4": rows4,
        })
    return in_maps


def kernel(x, conv_w, conv_b, gamma, beta, wq, wk, wv, wo):
    in_maps = make_in_maps(x, conv_w, conv_b, gamma, beta, wq, wk, wv, wo)
    nc = _get_nc()
    res = run_bass_kernel_spmd(nc, in_maps, core_ids=list(range(N_CORES)))
    out = np.concatenate([res.results[c]["yout"] for c in range(N_CORES)],
                         axis=0)
    return out.astype(np.float32)
